# revision 3
# baseline (speedup 1.0000x reference)
"""Trainium2 Bass kernel for nn_DeepSetsFunc (gnn_message_passing).

Reference computation (per set l of S=64 tokens, d=128 features):
    combined[l,j,:] = max_i( x[l,i,:] * (1 - eye)[i,j] )   # masked all-pairs max
    cm  = (relu(combined @ W1 + b1)) @ W2 + b2
    h   = (relu([x, cm] @ W3 + b3)) @ W4 + b4
    out = x + h

Sharding: data-parallel over L=256 sets across 8 cores (32 sets = 2048
tokens per core); weights replicated.

v2 design (vs the f32r baseline):
  * All elementwise stats run in bf16 on the DVE (2x tensor_tensor rate),
    with the two big SBUF-only multiplies offloaded to GPSIMD.
  * Masked all-pairs max via top-2 stats, select form:
      comb = ne*(relu(m1) - m2) + m2,  ne = (x < m1), m2 = max(0, 2nd max).
  * L1 matmul in bf16 (K=128, no DoubleRow possible); L2/L3/L4 in fp8e4
    with DoubleRow perf mode (2 K-subtiles contracted per instruction).
  * b2 is folded into b3 on the host (b3' = b3 + b2 @ W3[D:]), so the cm
    drain is a pure PSUM->fp8 copy.
  * Residual x rides the L4 accumulation as a bf16 identity matmul; b4 is
    applied in the final f32 drain. Host only transposes/concats.
"""

import sys

for p in ("/opt/trn_rl_repo", "/root/.axon_site/_ro/trn_rl_repo"):
    if p not in sys.path:
        sys.path.insert(0, p)

import ml_dtypes
import numpy as np

import concourse.bass as bass
import concourse.mybir as mybir
import concourse.tile as tile
from concourse import bacc
from concourse.bass_utils import run_bass_kernel_spmd

# Problem shapes (hardcoded per spec).
L, S, D = 256, 64, 128
NCORES = 8
LSH = L // NCORES          # 32 sets per core
NTOK = LSH * S             # 2048 tokens per core
D4 = 4 * D                 # 512
TT = 512                   # token tile; 8 sets per tile
NTT = NTOK // TT           # 4
SETS_TT = TT // S          # 8
N_WARMUP = 16              # PE warmup matmuls (HAM un-throttle)

F32 = mybir.dt.float32
BF16 = mybir.dt.bfloat16
F8 = mybir.dt.float8e4
DR = mybir.MatmulPerfMode.DoubleRow

_AX = mybir.AxisListType
_OP = mybir.AluOpType
_AF = mybir.ActivationFunctionType

NPBF = ml_dtypes.bfloat16
NPF8 = ml_dtypes.float8_e4m3   # TRN float8e4: max normal 240


def ts(i, size):
    return bass.ts(i, size)


def build_nc() -> bass.Bass:
    nc = bacc.Bacc("TRN2", target_bir_lowering=False, debug=False)

    xb_in = nc.dram_tensor("xb", [D, NTOK], BF16, kind="ExternalInput")
    x8_in = nc.dram_tensor("x8", [D, NTOK], F8, kind="ExternalInput")
    w1_in = nc.dram_tensor("w1", [D, D4], BF16, kind="ExternalInput")
    w2_in = nc.dram_tensor("w2", [D, 4, D], F8, kind="ExternalInput")
    w3_in = nc.dram_tensor("w3", [D, 2, 4, D], F8, kind="ExternalInput")
    w4_in = nc.dram_tensor("w4", [D, 4, D], F8, kind="ExternalInput")
    b1_in = nc.dram_tensor("b1s", [D, 4], F32, kind="ExternalInput")
    b3_in = nc.dram_tensor("b3s", [D, 4], F32, kind="ExternalInput")
    b4_in = nc.dram_tensor("b4s", [D, 1], F32, kind="ExternalInput")
    out = nc.dram_tensor("out", [D, NTOK], F32, kind="ExternalOutput")

    with tile.TileContext(nc) as tc:
        with (
            tc.tile_pool(name="const", bufs=1) as constp,
            tc.tile_pool(name="stat", bufs=2) as statp,
            tc.tile_pool(name="work", bufs=2) as workp,
            tc.tile_pool(name="psmm", bufs=8, space="PSUM") as psmm,
        ):
            # ---- warmup + constants ---------------------------------------
            zz = constp.tile([128, 128], BF16)
            nc.vector.memset(zz, 0.0)
            wps = psmm.tile([128, TT], F32, tag="mm", name="wps")
            for r in range(N_WARMUP):
                nc.tensor.matmul(
                    wps[:, :64], zz[:, :128], zz[:, :64], start=True, stop=True
                )

            # input DMAs: iteration 0 deps first, spread across queues
            xb = constp.tile([128, NTOK], BF16)
            nc.sync.dma_start(out=xb[:, ts(0, TT)], in_=xb_in[:, ts(0, TT)])
            w1s = constp.tile([128, D4], BF16)
            nc.scalar.dma_start(out=w1s, in_=w1_in[:, :])
            nc.scalar.dma_start(out=xb[:, ts(1, TT)], in_=xb_in[:, ts(1, TT)])
            w2s = constp.tile([128, 4, D], F8)
            nc.sync.dma_start(out=w2s, in_=w2_in[:, :, :])
            w3s = constp.tile([128, 2, 4, D], F8)
            nc.sync.dma_start(out=w3s, in_=w3_in[:, :, :, :])
            # xcm: DoubleRow moving pair for L3 — slice 0 = x fp8, slice 1 = cm
            xcm = constp.tile([128, 2, NTOK], F8)
            nc.scalar.dma_start(out=xcm[:, 0, :], in_=x8_in[:, :])
            w4s = constp.tile([128, 4, D], F8)
            nc.sync.dma_start(out=w4s, in_=w4_in[:, :, :])
            nc.sync.dma_start(out=xb[:, ts(2, TT)], in_=xb_in[:, ts(2, TT)])
            nc.scalar.dma_start(out=xb[:, ts(3, TT)], in_=xb_in[:, ts(3, TT)])

            b1s = constp.tile([128, 4], F32)
            nc.gpsimd.dma_start(out=b1s, in_=b1_in[:, :])
            b3s = constp.tile([128, 4], F32)
            nc.gpsimd.dma_start(out=b3s, in_=b3_in[:, :])
            b4s = constp.tile([128, 1], F32)
            nc.gpsimd.dma_start(out=b4s, in_=b4_in[:, :])

            # identity in bf16: residual x joins L4's accumulation on the PE
            from concourse.masks import make_identity
            identf = constp.tile([128, 128], F32)
            make_identity(nc, identf)
            identb = constp.tile([128, 128], BF16)
            nc.vector.tensor_copy(identb, identf)

            combs = [
                workp.tile([128, TT], BF16, tag="comb", name=f"comb_{i}")
                for i in range(NTT)
            ]

            def make_comb(tt_i, use_gp):
                """top-2 stats for 8 sets; select form, all bf16.

                comb = ne*(relu(m1) - m2) + m2 with ne = (x < m1),
                m2 = max(0, strict 2nd max). Exact when the per-(l,d) max
                is unique in bf16 (randn inputs: ~exact).
                """
                x3 = xb[:, ts(tt_i, TT)].rearrange("p (l s) -> p l s", s=S)
                m1 = statp.tile([128, SETS_TT], F32, tag="m1", name=f"m1_{tt_i}")
                nc.vector.tensor_reduce(m1, x3, axis=_AX.X, op=_OP.max)
                m1b = m1.unsqueeze(2).broadcast_to([128, SETS_TT, S])

                ne = workp.tile([128, TT], BF16, tag="ne", name=f"ne_{tt_i}")
                ne3 = ne.rearrange("p (l s) -> p l s", s=S)
                nc.vector.tensor_tensor(ne3, x3, m1b, op=_OP.is_lt)

                t2 = workp.tile([128, TT], BF16, tag="t2", name=f"t2_{tt_i}")
                t23 = t2.rearrange("p (l s) -> p l s", s=S)
                eng = nc.gpsimd if use_gp else nc.vector
                eng.tensor_tensor(t23, x3, ne3, op=_OP.mult)
                m2 = statp.tile([128, SETS_TT], F32, tag="m2", name=f"m2_{tt_i}")
                nc.vector.tensor_reduce(m2, t23, axis=_AX.X, op=_OP.max)

                # d2 = relu(m1) - m2  (tiny per-set stats)
                d2 = statp.tile([128, SETS_TT], F32, tag="d2", name=f"d2_{tt_i}")
                nc.vector.tensor_scalar(
                    d2, m1, 0.0, None, op0=_OP.max
                )
                nc.vector.tensor_tensor(d2, d2, m2, op=_OP.subtract)
                d2b = d2.unsqueeze(2).broadcast_to([128, SETS_TT, S])
                m2b = m2.unsqueeze(2).broadcast_to([128, SETS_TT, S])

                # X1 = ne * d2 (reuse t2 tile), comb = X1 + m2
                eng.tensor_tensor(t23, ne3, d2b, op=_OP.mult)
                comb3 = combs[tt_i].rearrange("p (l s) -> p l s", s=S)
                nc.vector.tensor_tensor(comb3, t23, m2b, op=_OP.add)

            make_comb(0, use_gp=False)
            make_comb(1, use_gp=True)

            for tt_i in range(NTT):
                cs = ts(tt_i, TT)
                comb = combs[tt_i]

                # ---- L1 (bf16): h1 = relu(W1.T @ comb + b1) --------------
                h1 = workp.tile([128, 4, TT], F8, tag="h1")
                for j in range(4):
                    ps = psmm.tile([128, TT], F32, tag="mm")
                    nc.tensor.matmul(
                        ps, w1s[:, ts(j, 128)], comb, start=True, stop=True
                    )
                    if j % 2 == 0:
                        nc.scalar.activation(
                            h1[:, j, :], ps, _AF.Relu, bias=b1s[:, j : j + 1]
                        )
                    else:
                        nc.vector.tensor_scalar(
                            h1[:, j, :], ps, b1s[:, j : j + 1], 0.0,
                            op0=_OP.add, op1=_OP.max,
                        )
                # next tiles' stats pipelined behind this tile's drains
                if tt_i + 2 < NTT:
                    make_comb(tt_i + 2, use_gp=True)
                # ---- L2 (fp8 DoubleRow): cm = W2.T @ h1 (b2 folded) ------
                ps2 = psmm.tile([128, TT], F32, tag="mm")
                for i in range(2):
                    nc.tensor.matmul(
                        ps2, w2s[:, 2 * i : 2 * i + 2, :],
                        h1[:, 2 * i : 2 * i + 2, :],
                        start=(i == 0), stop=(i == 1), perf_mode=DR,
                    )
                nc.vector.tensor_copy(xcm[:, 1, cs], ps2)
                # ---- L3 (fp8 DR): h3 = relu(W3.T @ [x, cm] + b3') --------
                h3 = workp.tile([128, 4, TT], F8, tag="h3")
                for j in range(4):
                    ps3 = psmm.tile([128, TT], F32, tag="mm")
                    nc.tensor.matmul(
                        ps3, w3s[:, :, j, :], xcm[:, :, cs],
                        start=True, stop=True, perf_mode=DR,
                    )
                    if j % 2 == 0:
                        nc.scalar.activation(
                            h3[:, j, :], ps3, _AF.Relu, bias=b3s[:, j : j + 1]
                        )
                    else:
                        nc.vector.tensor_scalar(
                            h3[:, j, :], ps3, b3s[:, j : j + 1], 0.0,
                            op0=_OP.add, op1=_OP.max,
                        )
                # ---- L4 (fp8 DR) + bf16 identity residual + b4 -----------
                ps4 = psmm.tile([128, TT], F32, tag="mm")
                for i in range(2):
                    nc.tensor.matmul(
                        ps4, w4s[:, 2 * i : 2 * i + 2, :],
                        h3[:, 2 * i : 2 * i + 2, :],
                        start=(i == 0), stop=False, perf_mode=DR,
                    )
                nc.tensor.matmul(
                    ps4, identb, xb[:, cs], start=False, stop=True
                )
                osb = workp.tile([128, TT], F32, tag="osb")
                nc.vector.tensor_scalar(osb, ps4, b4s, None, op0=_OP.add)
                dma_eng = nc.sync if tt_i % 2 == 0 else nc.scalar
                dma_eng.dma_start(out=out[:, cs], in_=osb)

    nc.compile()
    return nc


_NC_CACHE = None


def _prep_shared(inputs):
    f32 = np.float32
    W1 = np.ascontiguousarray(inputs["W1"], f32)
    W2 = np.ascontiguousarray(inputs["W2"], f32)
    W3 = np.ascontiguousarray(inputs["W3"], f32)
    W4 = np.ascontiguousarray(inputs["W4"], f32)
    b1 = np.asarray(inputs["b1"], f32)
    b2 = np.asarray(inputs["b2"], np.float64)
    b3 = np.asarray(inputs["b3"], np.float64)
    b4 = np.asarray(inputs["b4"], f32)
    b3p = (b3 + b2 @ W3[D:, :].astype(np.float64)).astype(f32)
    shared = {
        "w1": np.ascontiguousarray(W1.astype(NPBF)),
        "w2": np.ascontiguousarray(
            W2.reshape(4, D, D).transpose(1, 0, 2).astype(NPF8)
        ),
        "w3": np.ascontiguousarray(
            W3.reshape(2, D, 4, D).transpose(1, 0, 2, 3).astype(NPF8)
        ),
        "w4": np.ascontiguousarray(
            W4.reshape(4, D, D).transpose(1, 0, 2).astype(NPF8)
        ),
        "b1s": np.ascontiguousarray(b1.reshape(4, D).T),
        "b3s": np.ascontiguousarray(b3p.reshape(4, D).T),
        "b4s": np.ascontiguousarray(b4.reshape(D, 1)),
    }
    return shared


def make_in_maps(inputs):
    x = np.asarray(inputs["set_input"], dtype=np.float32)
    shared = _prep_shared(inputs)
    in_maps = []
    for c in range(NCORES):
        xt = x[c * LSH : (c + 1) * LSH].reshape(NTOK, D).T  # [D, NTOK]
        xt = np.ascontiguousarray(xt)
        in_maps.append(
            {
                "xb": xt.astype(NPBF),
                "x8": np.clip(xt, -240, 240).astype(NPF8),
                **shared,
            }
        )
    return in_maps


def kernel(**inputs) -> np.ndarray:
    global _NC_CACHE
    if _NC_CACHE is None:
        _NC_CACHE = build_nc()
    nc = _NC_CACHE

    in_maps = make_in_maps(inputs)
    res = run_bass_kernel_spmd(nc, in_maps, core_ids=list(range(NCORES)))
    outs = [
        res.results[c]["out"].T.reshape(LSH, S, D) for c in range(NCORES)
    ]
    return np.concatenate(outs, axis=0)


# revision 7
# speedup vs baseline: 1.0954x; 1.0954x over previous
"""Trainium2 Bass kernel for nn_DeepSetsFunc (gnn_message_passing).

Reference computation (per set l of S=64 tokens, d=128 features):
    combined[l,j,:] = max_i( x[l,i,:] * (1 - eye)[i,j] )   # masked all-pairs max
    cm  = (relu(combined @ W1 + b1)) @ W2 + b2
    h   = (relu([x, cm] @ W3 + b3)) @ W4 + b4
    out = x + h

Sharding: data-parallel over L=256 sets across 8 cores (32 sets = 2048
tokens per core); weights replicated.

v3 design (vs the f32r baseline):
  * Masked all-pairs max via per-set prefix/suffix running-max scans
    (tensor_tensor_scan, bf16): state = (r*state) max x, where the 0/1
    mask r resets the state to 0 at each set's first element -- the 0
    floor is exactly the mask's zero contribution. comb[j] =
    max(pfx[j-1], sfx[j+1]) (edges from the tiny strided columns).
    3 big DVE passes per tile instead of 7, and exact (no top-2 ties).
  * L1 matmul in bf16 (K=128, no DoubleRow possible); L2/L3/L4 in fp8e4
    with DoubleRow perf mode (2 K-subtiles contracted per instruction).
  * b2 is folded into b3 on the host (b3' = b3 + b2 @ W3[D:]), so the cm
    drain is a pure PSUM->fp8 copy.
  * Residual x rides the L4 accumulation as a bf16 identity matmul; b4 is
    applied in the final f32 drain. Host only transposes/concats.
"""

import sys

for p in ("/opt/trn_rl_repo", "/root/.axon_site/_ro/trn_rl_repo"):
    if p not in sys.path:
        sys.path.insert(0, p)

import ml_dtypes
import numpy as np

import concourse.bass as bass
import concourse.mybir as mybir
import concourse.tile as tile
from concourse import bacc
from concourse.bass_utils import run_bass_kernel_spmd

# Problem shapes (hardcoded per spec).
L, S, D = 256, 64, 128
NCORES = 8
LSH = L // NCORES          # 32 sets per core
NTOK = LSH * S             # 2048 tokens per core
D4 = 4 * D                 # 512
TT = 512                   # token tile; 8 sets per tile
NTT = NTOK // TT           # 4
SETS_TT = TT // S          # 8
N_WARMUP = 16              # PE warmup matmuls (HAM un-throttle)

F32 = mybir.dt.float32
BF16 = mybir.dt.bfloat16
F8 = mybir.dt.float8e4
DR = mybir.MatmulPerfMode.DoubleRow

_AX = mybir.AxisListType
_OP = mybir.AluOpType
_AF = mybir.ActivationFunctionType

NPBF = ml_dtypes.bfloat16
NPF8 = ml_dtypes.float8_e4m3   # TRN float8e4: max normal 240


def ts(i, size):
    return bass.ts(i, size)


def build_nc() -> bass.Bass:
    nc = bacc.Bacc("TRN2", target_bir_lowering=False, debug=False)

    xb_in = nc.dram_tensor("xb", [D, NTOK], BF16, kind="ExternalInput")
    x8_in = nc.dram_tensor("x8", [D, NTOK], F8, kind="ExternalInput")
    w1_in = nc.dram_tensor("w1", [D, D4], BF16, kind="ExternalInput")
    w2_in = nc.dram_tensor("w2", [D, 4, D], F8, kind="ExternalInput")
    w3_in = nc.dram_tensor("w3", [D, 2, 4, D], F8, kind="ExternalInput")
    w4_in = nc.dram_tensor("w4", [D, 4, D], F8, kind="ExternalInput")
    b1_in = nc.dram_tensor("b1s", [D, 4], F32, kind="ExternalInput")
    b3_in = nc.dram_tensor("b3s", [D, 4], F32, kind="ExternalInput")
    b4_in = nc.dram_tensor("b4s", [D, 1], F32, kind="ExternalInput")
    out = nc.dram_tensor("out", [D, NTOK], F32, kind="ExternalOutput")

    with tile.TileContext(nc) as tc:
        with (
            tc.tile_pool(name="const", bufs=1) as constp,
            tc.tile_pool(name="stat", bufs=2) as statp,
            tc.tile_pool(name="work", bufs=2) as workp,
            tc.tile_pool(name="psmm", bufs=8, space="PSUM") as psmm,
        ):
            # ---- tiny constants first: keep engine queues clear -----------
            zz = constp.tile([128, 128], BF16)
            nc.vector.memset(zz, 0.0)
            # scan reset masks: rA zero at each set's first token (forward
            # prefix scan), rB zero at each set's last token (reverse scan)
            rA = constp.tile([128, TT], BF16)
            nc.vector.memset(rA, 1.0)
            nc.vector.memset(
                rA.rearrange("p (l s) -> p l s", s=S)[:, :, 0:1], 0.0
            )
            rB = constp.tile([128, TT], BF16)
            nc.vector.memset(rB, 1.0)
            nc.vector.memset(
                rB.rearrange("p (l s) -> p l s", s=S)[:, :, S - 1 : S], 0.0
            )
            # identity in bf16: residual x joins L4's accumulation on the PE
            from concourse.masks import make_identity
            identf = constp.tile([128, 128], F32)
            make_identity(nc, identf)
            identb = constp.tile([128, 128], BF16)
            nc.vector.tensor_copy(identb, identf)

            # input DMAs: iteration 0 deps first, spread across queues
            xb = constp.tile([128, NTOK], BF16)
            nc.sync.dma_start(out=xb[:, ts(0, TT)], in_=xb_in[:, ts(0, TT)])
            w1s = constp.tile([128, D4], BF16)
            nc.scalar.dma_start(out=w1s, in_=w1_in[:, :])
            nc.sync.dma_start(out=xb[:, ts(1, TT)], in_=xb_in[:, ts(1, TT)])
            w2s = constp.tile([128, 4, D], F8)
            nc.scalar.dma_start(out=w2s, in_=w2_in[:, :, :])
            w3s = constp.tile([128, 2, 4, D], F8)
            nc.sync.dma_start(out=w3s, in_=w3_in[:, :, :, :])
            # xcm: DoubleRow moving pair for L3 — slice 0 = x fp8, slice 1 = cm
            xcm = constp.tile([128, 2, NTOK], F8)
            nc.scalar.dma_start(out=xcm[:, 0, :], in_=x8_in[:, :])
            w4s = constp.tile([128, 4, D], F8)
            nc.sync.dma_start(out=w4s, in_=w4_in[:, :, :])
            nc.scalar.dma_start(out=xb[:, ts(2, TT)], in_=xb_in[:, ts(2, TT)])
            nc.sync.dma_start(out=xb[:, ts(3, TT)], in_=xb_in[:, ts(3, TT)])

            b1s = constp.tile([128, 4], F32)
            nc.gpsimd.dma_start(out=b1s, in_=b1_in[:, :])
            b3s = constp.tile([128, 4], F32)
            nc.gpsimd.dma_start(out=b3s, in_=b3_in[:, :])
            b4s = constp.tile([128, 1], F32)
            nc.gpsimd.dma_start(out=b4s, in_=b4_in[:, :])

            combs = [
                workp.tile([128, TT], BF16, tag="comb", name=f"comb_{i}")
                for i in range(NTT)
            ]

            def make_comb(tt_i):
                """exact masked all-pairs max via 0-floored running-max
                scans: pfx[s] = max(0, x[..s]), sfx[s] = max(0, x[s..])
                per set (the r masks reset state to 0 at set boundaries),
                comb[j] = max(pfx[j-1], sfx[j+1])."""
                xt = xb[:, ts(tt_i, TT)]
                pfx = workp.tile([128, TT], BF16, tag="pfx", name=f"pfx_{tt_i}")
                nc.vector.tensor_tensor_scan(
                    pfx, rA, xt, 0.0, op0=_OP.mult, op1=_OP.max
                )
                sfx = workp.tile([128, TT], BF16, tag="sfx", name=f"sfx_{tt_i}")
                nc.vector.tensor_tensor_scan(
                    sfx[:, ::-1], rB[:, ::-1], xt[:, ::-1], 0.0,
                    op0=_OP.mult, op1=_OP.max,
                )
                comb3 = combs[tt_i].rearrange("p (l s) -> p l s", s=S)
                pfx3 = pfx.rearrange("p (l s) -> p l s", s=S)
                sfx3 = sfx.rearrange("p (l s) -> p l s", s=S)
                nc.vector.tensor_tensor(
                    comb3[:, :, 1 : S - 1], pfx3[:, :, 0 : S - 2],
                    sfx3[:, :, 2:S], op=_OP.max,
                )
                nc.vector.tensor_copy(comb3[:, :, 0:1], sfx3[:, :, 1:2])
                nc.vector.tensor_copy(
                    comb3[:, :, S - 1 : S], pfx3[:, :, S - 2 : S - 1]
                )

            make_comb(0)
            make_comb(1)
            # warmup train: runs while the DMAs/stats fill, so the HAM
            # window is still warm when the first real matmuls issue
            wps = psmm.tile([128, TT], F32, tag="mm", name="wps")
            for r in range(N_WARMUP):
                nc.tensor.matmul(
                    wps[:, :128], zz[:, :128], zz[:, :128],
                    start=True, stop=True,
                )

            for tt_i in range(NTT):
                cs = ts(tt_i, TT)
                comb = combs[tt_i]

                # ---- L1 (bf16): h1 = relu(W1.T @ comb + b1) --------------
                h1 = workp.tile([128, 4, TT], F8, tag="h1")
                for j in range(4):
                    ps = psmm.tile([128, TT], F32, tag="mm")
                    nc.tensor.matmul(
                        ps, w1s[:, ts(j, 128)], comb, start=True, stop=True
                    )
                    if j < 3:
                        nc.scalar.activation(
                            h1[:, j, :], ps, _AF.Relu, bias=b1s[:, j : j + 1]
                        )
                    else:
                        nc.vector.tensor_scalar(
                            h1[:, j, :], ps, b1s[:, j : j + 1], 0.0,
                            op0=_OP.add, op1=_OP.max,
                        )
                # next tiles' stats pipelined behind this tile's drains
                if tt_i + 2 < NTT:
                    make_comb(tt_i + 2)
                # ---- L2 (fp8 DoubleRow): cm = W2.T @ h1 (b2 folded) ------
                ps2 = psmm.tile([128, TT], F32, tag="mm")
                for i in range(2):
                    nc.tensor.matmul(
                        ps2, w2s[:, 2 * i : 2 * i + 2, :],
                        h1[:, 2 * i : 2 * i + 2, :],
                        start=(i == 0), stop=(i == 1), perf_mode=DR,
                    )
                nc.scalar.activation(xcm[:, 1, cs], ps2, _AF.Identity)
                # ---- L3 (fp8 DR): h3 = relu(W3.T @ [x, cm] + b3') --------
                h3 = workp.tile([128, 4, TT], F8, tag="h3")
                for j in range(4):
                    ps3 = psmm.tile([128, TT], F32, tag="mm")
                    nc.tensor.matmul(
                        ps3, w3s[:, :, j, :], xcm[:, :, cs],
                        start=True, stop=True, perf_mode=DR,
                    )
                    if j % 2 == 0:
                        nc.scalar.activation(
                            h3[:, j, :], ps3, _AF.Relu, bias=b3s[:, j : j + 1]
                        )
                    else:
                        nc.vector.tensor_scalar(
                            h3[:, j, :], ps3, b3s[:, j : j + 1], 0.0,
                            op0=_OP.add, op1=_OP.max,
                        )
                # ---- L4 (fp8 DR) + bf16 identity residual + b4 -----------
                ps4 = psmm.tile([128, TT], F32, tag="mm")
                for i in range(2):
                    nc.tensor.matmul(
                        ps4, w4s[:, 2 * i : 2 * i + 2, :],
                        h3[:, 2 * i : 2 * i + 2, :],
                        start=(i == 0), stop=False, perf_mode=DR,
                    )
                nc.tensor.matmul(
                    ps4, identb, xb[:, cs], start=False, stop=True
                )
                osb = workp.tile([128, TT], F32, tag="osb")
                nc.vector.tensor_scalar(osb, ps4, b4s, None, op0=_OP.add)
                dma_eng = nc.sync if tt_i % 2 == 0 else nc.scalar
                dma_eng.dma_start(out=out[:, cs], in_=osb)

    nc.compile()
    return nc


_NC_CACHE = None


def _prep_shared(inputs):
    f32 = np.float32
    W1 = np.ascontiguousarray(inputs["W1"], f32)
    W2 = np.ascontiguousarray(inputs["W2"], f32)
    W3 = np.ascontiguousarray(inputs["W3"], f32)
    W4 = np.ascontiguousarray(inputs["W4"], f32)
    b1 = np.asarray(inputs["b1"], f32)
    b2 = np.asarray(inputs["b2"], np.float64)
    b3 = np.asarray(inputs["b3"], np.float64)
    b4 = np.asarray(inputs["b4"], f32)
    b3p = (b3 + b2 @ W3[D:, :].astype(np.float64)).astype(f32)
    shared = {
        "w1": np.ascontiguousarray(W1.astype(NPBF)),
        "w2": np.ascontiguousarray(
            W2.reshape(4, D, D).transpose(1, 0, 2).astype(NPF8)
        ),
        "w3": np.ascontiguousarray(
            W3.reshape(2, D, 4, D).transpose(1, 0, 2, 3).astype(NPF8)
        ),
        "w4": np.ascontiguousarray(
            W4.reshape(4, D, D).transpose(1, 0, 2).astype(NPF8)
        ),
        "b1s": np.ascontiguousarray(b1.reshape(4, D).T),
        "b3s": np.ascontiguousarray(b3p.reshape(4, D).T),
        "b4s": np.ascontiguousarray(b4.reshape(D, 1)),
    }
    return shared


def make_in_maps(inputs):
    x = np.asarray(inputs["set_input"], dtype=np.float32)
    shared = _prep_shared(inputs)
    in_maps = []
    for c in range(NCORES):
        xt = x[c * LSH : (c + 1) * LSH].reshape(NTOK, D).T  # [D, NTOK]
        xt = np.ascontiguousarray(xt)
        in_maps.append(
            {
                "xb": xt.astype(NPBF),
                "x8": np.clip(xt, -240, 240).astype(NPF8),
                **shared,
            }
        )
    return in_maps


def kernel(**inputs) -> np.ndarray:
    global _NC_CACHE
    if _NC_CACHE is None:
        _NC_CACHE = build_nc()
    nc = _NC_CACHE

    in_maps = make_in_maps(inputs)
    res = run_bass_kernel_spmd(nc, in_maps, core_ids=list(range(NCORES)))
    outs = [
        res.results[c]["out"].T.reshape(LSH, S, D) for c in range(NCORES)
    ]
    return np.concatenate(outs, axis=0)


# revision 10
# speedup vs baseline: 1.1966x; 1.0924x over previous
"""Trainium2 Bass kernel for nn_DeepSetsFunc (gnn_message_passing).

Reference computation (per set l of S=64 tokens, d=128 features):
    combined[l,j,:] = max_i( x[l,i,:] * (1 - eye)[i,j] )   # masked all-pairs max
    cm  = (relu(combined @ W1 + b1)) @ W2 + b2
    h   = (relu([x, cm] @ W3 + b3)) @ W4 + b4
    out = x + h

Sharding: data-parallel over L=256 sets across 8 cores (32 sets = 2048
tokens per core); weights replicated.

v3 design (vs the f32r baseline):
  * Masked all-pairs max via per-set prefix/suffix running-max scans
    (tensor_tensor_scan, bf16): state = (r*state) max x, where the 0/1
    mask r resets the state to 0 at each set's first element -- the 0
    floor is exactly the mask's zero contribution. comb[j] =
    max(pfx[j-1], sfx[j+1]) (edges from the tiny strided columns).
    3 big DVE passes per tile instead of 7, and exact (no top-2 ties).
  * L1 matmul in bf16 (K=128, no DoubleRow possible); L2/L3/L4 in fp8e4
    with DoubleRow perf mode (2 K-subtiles contracted per instruction).
  * b2 is folded into b3 on the host (b3' = b3 + b2 @ W3[D:]), so the cm
    drain is a pure PSUM->fp8 copy.
  * Residual x rides the L4 accumulation as a bf16 identity matmul; b4 is
    applied in the final f32 drain. Host only transposes/concats.
"""

import sys

for p in ("/opt/trn_rl_repo", "/root/.axon_site/_ro/trn_rl_repo"):
    if p not in sys.path:
        sys.path.insert(0, p)

import ml_dtypes
import numpy as np

import concourse.bass as bass
import concourse.mybir as mybir
import concourse.tile as tile
from concourse import bacc
from concourse.bass_utils import run_bass_kernel_spmd

# Problem shapes (hardcoded per spec).
L, S, D = 256, 64, 128
NCORES = 8
LSH = L // NCORES          # 32 sets per core
NTOK = LSH * S             # 2048 tokens per core
D4 = 4 * D                 # 512
TT = 512                   # token tile; 8 sets per tile
NTT = NTOK // TT           # 4
SETS_TT = TT // S          # 8
N_WARMUP = 16              # PE warmup matmuls (HAM un-throttle)

F32 = mybir.dt.float32
BF16 = mybir.dt.bfloat16
F8 = mybir.dt.float8e4
DR = mybir.MatmulPerfMode.DoubleRow

_AX = mybir.AxisListType
_OP = mybir.AluOpType
_AF = mybir.ActivationFunctionType

NPBF = ml_dtypes.bfloat16
NPF8 = ml_dtypes.float8_e4m3   # TRN float8e4: max normal 240


def ts(i, size):
    return bass.ts(i, size)


def build_nc() -> bass.Bass:
    nc = bacc.Bacc("TRN2", target_bir_lowering=False, debug=False)

    xb_in = nc.dram_tensor("xb", [D, NTOK], BF16, kind="ExternalInput")
    x8_in = nc.dram_tensor("x8", [D, NTOK], F8, kind="ExternalInput")
    w1_in = nc.dram_tensor("w1", [D, D4], BF16, kind="ExternalInput")
    w2_in = nc.dram_tensor("w2", [D, 4, D], F8, kind="ExternalInput")
    w3_in = nc.dram_tensor("w3", [D, 2, 4, D], F8, kind="ExternalInput")
    w4_in = nc.dram_tensor("w4", [D, 4, D], F8, kind="ExternalInput")
    b1_in = nc.dram_tensor("b1s", [D, 4], F32, kind="ExternalInput")
    b3_in = nc.dram_tensor("b3s", [D, 4], F32, kind="ExternalInput")
    b4_in = nc.dram_tensor("b4s", [D, 1], F32, kind="ExternalInput")
    out = nc.dram_tensor("out", [D, NTOK], F32, kind="ExternalOutput")

    with tile.TileContext(nc) as tc:
        with (
            tc.tile_pool(name="const", bufs=1) as constp,
            tc.tile_pool(name="stat", bufs=2) as statp,
            tc.tile_pool(name="work", bufs=2) as workp,
            tc.tile_pool(name="psmm", bufs=8, space="PSUM") as psmm,
        ):
            # ---- tiny constants first: keep engine queues clear -----------
            zz = constp.tile([128, 128], BF16)
            nc.vector.memset(zz, 0.0)
            # scan reset masks: rA zero at each set's first token (forward
            # prefix scan), rB zero at each set's last token (reverse scan)
            rA = constp.tile([128, TT], BF16)
            nc.vector.memset(rA, 1.0)
            nc.vector.memset(
                rA.rearrange("p (l s) -> p l s", s=S)[:, :, 0:1], 0.0
            )
            rB = constp.tile([128, TT], BF16)
            nc.vector.memset(rB, 1.0)
            nc.vector.memset(
                rB.rearrange("p (l s) -> p l s", s=S)[:, :, S - 1 : S], 0.0
            )
            # identity in bf16: residual x joins L4's accumulation on the PE
            from concourse.masks import make_identity
            identf = constp.tile([128, 128], F32)
            make_identity(nc, identf)
            identb = constp.tile([128, 128], BF16)
            nc.vector.tensor_copy(identb, identf)

            # input DMAs: iteration 0 deps first, spread across queues
            xb = constp.tile([128, NTOK], BF16)
            nc.sync.dma_start(out=xb[:, ts(0, TT)], in_=xb_in[:, ts(0, TT)])
            w1s = constp.tile([128, D4], BF16)
            nc.scalar.dma_start(out=w1s, in_=w1_in[:, :])
            nc.sync.dma_start(out=xb[:, ts(1, TT)], in_=xb_in[:, ts(1, TT)])
            w2s = constp.tile([128, 4, D], F8)
            nc.scalar.dma_start(out=w2s, in_=w2_in[:, :, :])
            w3s = constp.tile([128, 2, 4, D], F8)
            nc.sync.dma_start(out=w3s, in_=w3_in[:, :, :, :])
            # xcm: DoubleRow moving pair for L3 — slice 0 = x fp8, slice 1 = cm
            xcm = constp.tile([128, 2, NTOK], F8)
            nc.scalar.dma_start(out=xcm[:, 0, :], in_=x8_in[:, :])
            w4s = constp.tile([128, 4, D], F8)
            nc.sync.dma_start(out=w4s, in_=w4_in[:, :, :])
            nc.scalar.dma_start(out=xb[:, ts(2, TT)], in_=xb_in[:, ts(2, TT)])
            nc.sync.dma_start(out=xb[:, ts(3, TT)], in_=xb_in[:, ts(3, TT)])

            b1s = constp.tile([128, 4], F32)
            nc.gpsimd.dma_start(out=b1s, in_=b1_in[:, :])
            b3s = constp.tile([128, 4], F32)
            nc.gpsimd.dma_start(out=b3s, in_=b3_in[:, :])
            b4s = constp.tile([128, 1], F32)
            nc.gpsimd.dma_start(out=b4s, in_=b4_in[:, :])

            combs = [
                workp.tile([128, TT], BF16, tag="comb", name=f"comb_{i}")
                for i in range(NTT)
            ]

            def make_comb_scan(tt_i):
                """exact masked all-pairs max via 0-floored running-max
                scans (DVE-only, shortest serial latency — used for the
                front tiles): comb[j] = max(pfx[j-1], sfx[j+1])."""
                xt = xb[:, ts(tt_i, TT)]
                pfx = workp.tile([128, TT], BF16, tag="pfx", name=f"pfx_{tt_i}")
                nc.vector.tensor_tensor_scan(
                    pfx, rA, xt, 0.0, op0=_OP.mult, op1=_OP.max
                )
                sfx = workp.tile([128, TT], BF16, tag="sfx", name=f"sfx_{tt_i}")
                nc.vector.tensor_tensor_scan(
                    sfx[:, ::-1], rB[:, ::-1], xt[:, ::-1], 0.0,
                    op0=_OP.mult, op1=_OP.max,
                )
                comb3 = combs[tt_i].rearrange("p (l s) -> p l s", s=S)
                pfx3 = pfx.rearrange("p (l s) -> p l s", s=S)
                sfx3 = sfx.rearrange("p (l s) -> p l s", s=S)
                nc.vector.tensor_tensor(
                    comb3[:, :, 1 : S - 1], pfx3[:, :, 0 : S - 2],
                    sfx3[:, :, 2:S], op=_OP.max,
                )
                nc.vector.tensor_copy(comb3[:, :, 0:1], sfx3[:, :, 1:2])
                nc.vector.tensor_copy(
                    comb3[:, :, S - 1 : S], pfx3[:, :, S - 2 : S - 1]
                )

            def make_comb_gp(tt_i):
                """top-2 stats with the three big SBUF passes on GPSIMD
                (frees the DVE for PSUM drains): comb = ne*(c1-m2) + m2."""
                x3 = xb[:, ts(tt_i, TT)].rearrange("p (l s) -> p l s", s=S)
                m1 = statp.tile([128, SETS_TT], BF16, tag="m1", name=f"m1_{tt_i}")
                nc.vector.tensor_reduce(m1, x3, axis=_AX.X, op=_OP.max)
                m1b = m1.unsqueeze(2).broadcast_to([128, SETS_TT, S])
                ne = workp.tile([128, TT], BF16, tag="ne", name=f"ne_{tt_i}")
                ne3 = ne.rearrange("p (l s) -> p l s", s=S)
                nc.vector.tensor_tensor(ne3, x3, m1b, op=_OP.is_lt)
                t2 = workp.tile([128, TT], BF16, tag="t2", name=f"t2_{tt_i}")
                t23 = t2.rearrange("p (l s) -> p l s", s=S)
                nc.gpsimd.tensor_tensor(t23, x3, ne3, op=_OP.mult)
                m2 = statp.tile([128, SETS_TT], BF16, tag="m2", name=f"m2_{tt_i}")
                nc.vector.tensor_reduce(m2, t23, axis=_AX.X, op=_OP.max)
                d2 = statp.tile([128, SETS_TT], BF16, tag="d2", name=f"d2_{tt_i}")
                nc.vector.tensor_scalar(d2, m1, 0.0, None, op0=_OP.max)
                nc.vector.tensor_tensor(d2, d2, m2, op=_OP.subtract)
                d2b = d2.unsqueeze(2).broadcast_to([128, SETS_TT, S])
                m2b = m2.unsqueeze(2).broadcast_to([128, SETS_TT, S])
                nc.gpsimd.tensor_tensor(t23, ne3, d2b, op=_OP.mult)
                comb3 = combs[tt_i].rearrange("p (l s) -> p l s", s=S)
                nc.vector.tensor_tensor(comb3, t23, m2b, op=_OP.add)

            make_comb_scan(0)
            make_comb_scan(1)
            make_comb_gp(2)
            # warmup train: gated on the first x DMA so it bridges the
            # stats window and the PE is still warm at the first real mm
            wps = psmm.tile([128, TT], F32, tag="mm", name="wps")
            for r in range(N_WARMUP):
                nc.tensor.matmul(
                    wps[:, :128], zz[:, :128], xb[:, :128],
                    start=True, stop=True,
                )

            def l1_mms(i):
                for j in range(4):
                    ps = psmm.tile([128, TT], F32, tag="mm", name=f"ps1_{i}_{j}")
                    nc.tensor.matmul(
                        ps, w1s[:, ts(j, 128)], combs[i], start=True, stop=True
                    )
                    l1ps[i].append(ps)

            l1ps = [[] for _ in range(NTT)]
            l1_mms(0)

            for tt_i in range(NTT):
                cs = ts(tt_i, TT)
                # ---- h1 drains: ACT j0-j2, DVE j3 ------------------------
                h1 = workp.tile([128, 4, TT], F8, tag="h1")
                for j in range(4):
                    ps = l1ps[tt_i][j]
                    if j < 3:
                        nc.scalar.activation(
                            h1[:, j, :], ps, _AF.Relu, bias=b1s[:, j : j + 1]
                        )
                    else:
                        nc.vector.tensor_scalar(
                            h1[:, j, :], ps, b1s[:, j : j + 1], 0.0,
                            op0=_OP.add, op1=_OP.max,
                        )
                # ---- L2 (fp8 DR) -----------------------------------------
                ps2 = psmm.tile([128, TT], F32, tag="mm")
                for i in range(2):
                    nc.tensor.matmul(
                        ps2, w2s[:, 2 * i : 2 * i + 2, :],
                        h1[:, 2 * i : 2 * i + 2, :],
                        start=(i == 0), stop=(i == 1), perf_mode=DR,
                    )
                # next tile's L1 fills the PE while cm drains
                if tt_i + 1 < NTT:
                    l1_mms(tt_i + 1)
                if tt_i + 3 < NTT:
                    make_comb_gp(tt_i + 3)
                # cm drain (pure copy, b2 folded into b3')
                nc.scalar.activation(xcm[:, 1, cs], ps2, _AF.Identity)
                # ---- L3 (fp8 DR): h3 = relu(W3.T @ [x, cm] + b3') --------
                h3 = workp.tile([128, 4, TT], F8, tag="h3")
                ps3s = []
                for j in range(4):
                    ps3 = psmm.tile([128, TT], F32, tag="mm")
                    nc.tensor.matmul(
                        ps3, w3s[:, :, j, :], xcm[:, :, cs],
                        start=True, stop=True, perf_mode=DR,
                    )
                    ps3s.append(ps3)
                for j in range(4):
                    if j < 2:
                        nc.scalar.activation(
                            h3[:, j, :], ps3s[j], _AF.Relu,
                            bias=b3s[:, j : j + 1],
                        )
                    else:
                        nc.vector.tensor_scalar(
                            h3[:, j, :], ps3s[j], b3s[:, j : j + 1], 0.0,
                            op0=_OP.add, op1=_OP.max,
                        )
                # ---- L4 (fp8 DR) + bf16 identity residual + b4 -----------
                ps4 = psmm.tile([128, TT], F32, tag="mm")
                for i in range(2):
                    nc.tensor.matmul(
                        ps4, w4s[:, 2 * i : 2 * i + 2, :],
                        h3[:, 2 * i : 2 * i + 2, :],
                        start=(i == 0), stop=False, perf_mode=DR,
                    )
                nc.tensor.matmul(
                    ps4, identb, xb[:, cs], start=False, stop=True
                )
                osb = workp.tile([128, TT], F32, tag="osb")
                nc.vector.tensor_scalar(osb, ps4, b4s, None, op0=_OP.add)
                dma_eng = nc.sync if tt_i % 2 == 0 else nc.scalar
                dma_eng.dma_start(out=out[:, cs], in_=osb)

    nc.compile()
    return nc


_NC_CACHE = None


def _prep_shared(inputs):
    f32 = np.float32
    W1 = np.ascontiguousarray(inputs["W1"], f32)
    W2 = np.ascontiguousarray(inputs["W2"], f32)
    W3 = np.ascontiguousarray(inputs["W3"], f32)
    W4 = np.ascontiguousarray(inputs["W4"], f32)
    b1 = np.asarray(inputs["b1"], f32)
    b2 = np.asarray(inputs["b2"], np.float64)
    b3 = np.asarray(inputs["b3"], np.float64)
    b4 = np.asarray(inputs["b4"], f32)
    b3p = (b3 + b2 @ W3[D:, :].astype(np.float64)).astype(f32)
    shared = {
        "w1": np.ascontiguousarray(W1.astype(NPBF)),
        "w2": np.ascontiguousarray(
            W2.reshape(4, D, D).transpose(1, 0, 2).astype(NPF8)
        ),
        "w3": np.ascontiguousarray(
            W3.reshape(2, D, 4, D).transpose(1, 0, 2, 3).astype(NPF8)
        ),
        "w4": np.ascontiguousarray(
            W4.reshape(4, D, D).transpose(1, 0, 2).astype(NPF8)
        ),
        "b1s": np.ascontiguousarray(b1.reshape(4, D).T),
        "b3s": np.ascontiguousarray(b3p.reshape(4, D).T),
        "b4s": np.ascontiguousarray(b4.reshape(D, 1)),
    }
    return shared


def make_in_maps(inputs):
    x = np.asarray(inputs["set_input"], dtype=np.float32)
    shared = _prep_shared(inputs)
    in_maps = []
    for c in range(NCORES):
        xt = x[c * LSH : (c + 1) * LSH].reshape(NTOK, D).T  # [D, NTOK]
        xt = np.ascontiguousarray(xt)
        in_maps.append(
            {
                "xb": xt.astype(NPBF),
                "x8": np.clip(xt, -240, 240).astype(NPF8),
                **shared,
            }
        )
    return in_maps


def kernel(**inputs) -> np.ndarray:
    global _NC_CACHE
    if _NC_CACHE is None:
        _NC_CACHE = build_nc()
    nc = _NC_CACHE

    in_maps = make_in_maps(inputs)
    res = run_bass_kernel_spmd(nc, in_maps, core_ids=list(range(NCORES)))
    outs = [
        res.results[c]["out"].T.reshape(LSH, S, D) for c in range(NCORES)
    ]
    return np.concatenate(outs, axis=0)
